# revision 14
# baseline (speedup 1.0000x reference)
"""GRUAggregation1d Trainium2 kernel.

Computes, for xs [B=16, 512, L=8192], z_prev [B, 128, L] (all fp32):
    q  = sigmoid(Wq@xs + Uq@z + bq)        (per position l, batch b)
    r  = sigmoid(Wr@xs + Ur@z + br)
    zt = tanh(Wz@xs + Uz@(r*z) + bz)
    out = q*z + (1-q)*zt

Sharding: data-parallel over batch, 8 cores x 2 batches. Per core: 32
position-tiles of 512; 15 bf16 matmuls per tile (4 W-chunks + 1 U per
gate), PSUM-accumulated, sigmoid/tanh on ScalarE with fused per-gate
bias, bf16 combine on VectorE.

fp8 DoubleRow was evaluated and rejected on measurement: DR executes at
1 col/cycle (213ns per 512-col matmul, 2x contraction depth) but
occupies BOTH PE weight buffers, so LDWEIGHTS cannot overlap any DR
execution; the exposed ~136ns loads cancel the arithmetic gain, and
e4m3 weight noise alone is ~2.7e-2 max-rel error without residual
matmuls. bf16 matmuls at 213ns with fully-hidden 95ns weight loads are
the per-tile floor (~3.2us).

vs the original baseline (157us -> ~120us):
  - DMA diet: z_prev and the output travel as bf16 (host casts):
    16 MiB xs + 4 z + 4 out per core instead of 16+8+8.
  - DMA batching: host packs xs+z into one 5120B row per
    (partition, tile); ONE input DMA per 2 tiles and one output DMA per
    2 tiles (on the Activation HWDGE queue; inputs on the SP queue),
    amortizing the ~625ns HWDGE descriptor-generation serialization.
  - Startup: tile 0/1 inputs arrive as xs-only + z-only DMAs with the
    first weight slice ([Wq0,Wq1]) raced in between, so the first
    matmul fires ~3us earlier than with monolithic transfers.
  - Drain: the last tile's tanh+combine+store run in two half-tiles so
    the Scalar->Vector->DMA tail pipelines instead of serializing.
  - All elementwise work is bf16 (DVE 2x mode); the z->bf16 ScalarE
    cast of the baseline is gone.
"""

from contextlib import ExitStack

import ml_dtypes
import numpy as np

import concourse.bass as bass
import concourse.mybir as mybir
import concourse.tile as tile
from concourse import bacc
from concourse.bass_utils import run_bass_kernel_spmd

B, IN_DIM, WIDTH, L = 16, 512, 128, 8192
N_CORES = 8
B_PER = B // N_CORES          # batches per core
KC = IN_DIM // 128            # K chunks for the W matmuls
NT = 512                      # positions per tile
N_LT = L // NT                # position tiles per batch
T = B_PER * N_LT              # tiles per core
SS = 2                        # tiles per DMA superstep
XS_B = KC * NT * 2            # bf16 xs bytes per row
Z_B = 2 * NT                  # bf16 z bytes per row
ROW = XS_B + Z_B              # input row bytes per (partition, tile)

F32 = mybir.dt.float32
BF16 = mybir.dt.bfloat16
U8 = mybir.dt.uint8

_module_cache = {}


def _build():
    key = ("bf16v5", NT, SS)
    if key in _module_cache:
        return _module_cache[key]

    nc = bacc.Bacc("TRN2", target_bir_lowering=False, debug=False,
                   num_devices=N_CORES)

    inp_d = nc.dram_tensor("inp", [128, T, ROW], U8, kind="ExternalInput").ap()
    w_d = nc.dram_tensor("wall", [128, 15, 128], BF16,
                         kind="ExternalInput").ap()
    b_d = nc.dram_tensor("ball", [128, 3], F32, kind="ExternalInput").ap()
    out_d = nc.dram_tensor("out", [128, T, NT], BF16,
                           kind="ExternalOutput").ap()

    with tile.TileContext(nc) as tc, ExitStack() as ctx:
        wpool = ctx.enter_context(tc.tile_pool(name="weights", bufs=1))
        io = ctx.enter_context(tc.tile_pool(name="io", bufs=4))
        sig = ctx.enter_context(tc.tile_pool(name="sig", bufs=3))
        acts = ctx.enter_context(tc.tile_pool(name="acts", bufs=2))
        ost_p = ctx.enter_context(tc.tile_pool(name="ost", bufs=3))
        ps_q = ctx.enter_context(tc.tile_pool(name="ps_q", bufs=2,
                                              space="PSUM"))
        ps_r = ctx.enter_context(tc.tile_pool(name="ps_r", bufs=2,
                                              space="PSUM"))
        ps_zt = ctx.enter_context(tc.tile_pool(name="ps_zt", bufs=3,
                                               space="PSUM"))
        ps_wu = ctx.enter_context(tc.tile_pool(name="ps_wu", bufs=1,
                                               space="PSUM"))

        wall = wpool.tile([128, 15, 128], BF16, tag="wall")
        ball = wpool.tile([128, 3], F32, tag="ball")

        # PE p-state warmup: the PE runs its first ~3us at 1.2GHz. Burn
        # dummy matmuls on scratch tiles while the startup DMAs are in
        # flight so the first real matmul runs at full clock.
        scr_w = wpool.tile([128, 128], BF16, tag="scr_w")
        scr_x = wpool.tile([128, NT], BF16, tag="scr_x")
        nc.gpsimd.memset(scr_w[:], 0.0)
        nc.gpsimd.memset(scr_x[:], 0.0)
        wu_ps = ps_wu.tile([128, NT], F32, tag="wu")
        for i in range(8):
            nc.tensor.matmul(wu_ps[:], scr_w[:], scr_x[:],
                             start=(i == 0), stop=(i == 7))

        # SP queue: the leading weight slice [Wq0, Wq1] first (small),
        # then tile-0/1 xs. Activation queue (in parallel): z parts, the
        # remaining weights, biases. First MATMUL only needs tile-0 xs +
        # [Wq0, Wq1].
        nc.sync.dma_start(wall[:, 0:2, :], w_d[:, 0:2, :])
        warm = []
        for t in range(2):
            wt = io.tile([128, 1, ROW], U8, tag="io_w", name="warm")
            nc.sync.dma_start(wt[:, :, 0:XS_B], inp_d[:, t:t + 1, 0:XS_B])
            nc.scalar.dma_start(wt[:, :, XS_B:ROW],
                                inp_d[:, t:t + 1, XS_B:ROW])
            if t == 0:
                nc.scalar.dma_start(wall[:, 2:15, :], w_d[:, 2:15, :])
                nc.scalar.dma_start(ball[:], b_d[:])
            warm.append(wt)

        carry = None
        ost = [None]

        def finish(c):
            """Trailing half of tile tc_ (Uz@(rz) matmul, tanh, combine,
            store), emitted during tile tc_+1 (after the loop for the
            last tile, split in halves to pipeline the drain)."""
            zt_ps, q_s, rz, z_v, tc_ = c
            nc.tensor.matmul(zt_ps[:], wall[:, 14, :], rz[:],
                             start=False, stop=True)
            sc, jc = divmod(tc_, SS)
            if jc == 0:
                ost[0] = ost_p.tile([128, SS, NT], BF16, tag="ost",
                                    name="ost")
            if tc_ < T - 1:
                halves = ((0, NT),)
            else:
                halves = tuple((h * NT // 4, (h + 1) * NT // 4)
                               for h in range(4))
            for h0, h1 in halves:
                zt_s = acts.tile([128, h1 - h0], BF16, tag=f"zt_s{h0}",
                                 name="zt_s")
                nc.scalar.activation(zt_s[:], zt_ps[:, h0:h1],
                                     mybir.ActivationFunctionType.Tanh,
                                     bias=ball[:, 2:3])
                # out = zt + q*(z - zt)
                diff = acts.tile([128, h1 - h0], BF16, tag=f"diff{h0}",
                                 name="diff")
                nc.vector.tensor_sub(diff[:], z_v[:, h0:h1], zt_s[:])
                prod = acts.tile([128, h1 - h0], BF16, tag=f"prod{h0}",
                                 name="prod")
                nc.vector.tensor_mul(prod[:], q_s[:, h0:h1], diff[:])
                nc.vector.tensor_add(ost[0][:, jc, h0:h1], zt_s[:], prod[:])
                if tc_ >= T - 2:
                    # drain: ship each piece as soon as it is ready
                    nc.scalar.dma_start(
                        out_d[:, tc_:tc_ + 1, h0:h1],
                        ost[0][:, jc:jc + 1, h0:h1])
            if jc == SS - 1 and tc_ < T - 2:
                # out-DMAs ride the Activation HWDGE queue so the SP queue
                # only carries input DMAs
                nc.scalar.dma_start(out_d[:, SS * sc:SS * (sc + 1), :],
                                    ost[0][:])

        io_t = None
        for t in range(T):
            s, j = divmod(t, SS)
            if t < 2:
                cur, cj = warm[t], 0
            else:
                if j == 0:
                    io_t = io.tile([128, SS, ROW], U8, tag="io")
                    nc.sync.dma_start(io_t[:],
                                      inp_d[:, SS * s:SS * (s + 1), :])
                cur, cj = io_t, j
            xs_v = cur[:, cj, 0:XS_B].bitcast(BF16).rearrange(
                "p (k n) -> p k n", k=KC)
            z_v = cur[:, cj, XS_B:ROW].bitcast(BF16)

            q_ps = ps_q.tile([128, NT], F32, tag="q")
            for k in range(KC):
                nc.tensor.matmul(q_ps[:], wall[:, k, :], xs_v[:, k, :],
                                 start=(k == 0), stop=False)
            nc.tensor.matmul(q_ps[:], wall[:, 4, :], z_v,
                             start=False, stop=True)
            r_ps = ps_r.tile([128, NT], F32, tag="r")
            for k in range(KC):
                nc.tensor.matmul(r_ps[:], wall[:, 5 + k, :], xs_v[:, k, :],
                                 start=(k == 0), stop=False)
            nc.tensor.matmul(r_ps[:], wall[:, 9, :], z_v,
                             start=False, stop=True)

            if carry is not None:
                finish(carry)
                carry = None

            zt_ps = ps_zt.tile([128, NT], F32, tag="zt")
            for k in range(KC):
                nc.tensor.matmul(zt_ps[:], wall[:, 10 + k, :], xs_v[:, k, :],
                                 start=(k == 0), stop=False)

            q_s = sig.tile([128, NT], BF16, tag="q_s")
            nc.scalar.activation(q_s[:], q_ps[:],
                                 mybir.ActivationFunctionType.Sigmoid,
                                 bias=ball[:, 0:1])
            r_s = acts.tile([128, NT], BF16, tag="r_s")
            nc.scalar.activation(r_s[:], r_ps[:],
                                 mybir.ActivationFunctionType.Sigmoid,
                                 bias=ball[:, 1:2])
            rz = acts.tile([128, NT], BF16, tag="rz")
            nc.vector.tensor_mul(rz[:], r_s[:], z_v)

            carry = (zt_ps, q_s, rz, z_v, t)

        finish(carry)

    nc.compile()
    _module_cache[key] = nc
    return nc


def _pack_inputs(xs, zp):
    """Per-core [128, T, ROW] uint8 rows: 4096B bf16 xs + 1024B bf16 z."""
    xsb = xs.astype(ml_dtypes.bfloat16)
    v = xsb.reshape(B, KC, 128, N_LT, NT)        # [b, k, p, i, n]
    v = np.ascontiguousarray(v.transpose(0, 2, 3, 1, 4))  # [b, p, i, k, n]
    v = v.reshape(B, 128, N_LT, KC * NT).view(np.uint8)
    zb = zp.astype(ml_dtypes.bfloat16).reshape(B, 128, N_LT, NT)
    zb = zb.view(np.uint8).reshape(B, 128, N_LT, Z_B)
    rows = np.concatenate([v, zb], axis=-1)      # [b, p, i, ROW]
    cores = []
    for c in range(N_CORES):
        rc = rows[c * B_PER:(c + 1) * B_PER]
        rc = rc.transpose(1, 0, 2, 3).reshape(128, T, ROW)
        cores.append(np.ascontiguousarray(rc))
    return cores


def _pack_weights(inputs):
    wall = np.zeros((128, 15, 128), dtype=np.float32)
    ball = np.zeros((128, 3), dtype=np.float32)
    for g, (wn, un, wbn, ubn) in enumerate((
            ("Wq_w", "Uq_w", "Wq_b", "Uq_b"),
            ("Wr_w", "Ur_w", "Wr_b", "Ur_b"),
            ("Wz_w", "Uz_w", "Wz_b", "Uz_b"))):
        W = np.asarray(inputs[wn], dtype=np.float32)         # [128, 512]
        U = np.asarray(inputs[un], dtype=np.float32)         # [128, 128]
        Wt = W.T.reshape(KC, 128, 128)                       # [k, p, m]
        for k in range(KC):
            wall[:, 5 * g + k, :] = Wt[k]
        wall[:, 5 * g + 4, :] = U.T
        ball[:, g] = (np.asarray(inputs[wbn], dtype=np.float32)
                      + np.asarray(inputs[ubn], dtype=np.float32))
    return (np.ascontiguousarray(wall.astype(ml_dtypes.bfloat16)),
            np.ascontiguousarray(ball))


def _run(inputs, trace=False, **run_kwargs):
    xs = np.asarray(inputs["xs"], dtype=np.float32)
    zp = np.ascontiguousarray(np.asarray(inputs["z_prev"], dtype=np.float32))
    assert xs.shape == (B, IN_DIM, L) and zp.shape == (B, WIDTH, L)

    inp_cores = _pack_inputs(xs, zp)
    wall, ball = _pack_weights(inputs)

    nc = _build()
    in_maps = [{"inp": inp_cores[c], "wall": wall, "ball": ball}
               for c in range(N_CORES)]

    res = run_bass_kernel_spmd(nc, in_maps, core_ids=list(range(N_CORES)),
                               trace=trace, **run_kwargs)
    outs = []
    for c in range(N_CORES):
        o = res.results[c]["out"]                 # [128, T, NT] bf16
        o = np.asarray(o).reshape(128, B_PER, N_LT, NT)
        o = o.transpose(1, 0, 2, 3).reshape(B_PER, 128, L)
        outs.append(o.astype(np.float32))
    out = np.concatenate(outs, axis=0)
    return out, res


def kernel(**inputs):
    out, _ = _run(inputs, trace=False)
    return out


# revision 15
# speedup vs baseline: 1.0125x; 1.0125x over previous
"""GRUAggregation1d Trainium2 kernel.

Computes, for xs [B=16, 512, L=8192], z_prev [B, 128, L] (all fp32):
    q  = sigmoid(Wq@xs + Uq@z + bq)        (per position l, batch b)
    r  = sigmoid(Wr@xs + Ur@z + br)
    zt = tanh(Wz@xs + Uz@(r*z) + bz)
    out = q*z + (1-q)*zt

Sharding: data-parallel over batch, 8 cores x 2 batches. Per core: 32
position-tiles of 512; 15 bf16 matmuls per tile (4 W-chunks + 1 U per
gate), PSUM-accumulated, sigmoid/tanh on ScalarE with fused per-gate
bias, bf16 combine on VectorE.

fp8 DoubleRow was evaluated and rejected on measurement: DR executes at
1 col/cycle (213ns per 512-col matmul, 2x contraction depth) but
occupies BOTH PE weight buffers, so LDWEIGHTS cannot overlap any DR
execution; the exposed ~136ns loads cancel the arithmetic gain, and
e4m3 weight noise alone is ~2.7e-2 max-rel error without residual
matmuls. bf16 matmuls at 213ns with fully-hidden 95ns weight loads are
the per-tile floor (~3.2us).

vs the original baseline (157us -> ~120us):
  - DMA diet: z_prev and the output travel as bf16 (host casts):
    16 MiB xs + 4 z + 4 out per core instead of 16+8+8.
  - DMA batching: host packs xs+z into one 5120B row per
    (partition, tile); ONE input DMA per 2 tiles and one output DMA per
    2 tiles (on the Activation HWDGE queue; inputs on the SP queue),
    amortizing the ~625ns HWDGE descriptor-generation serialization.
  - Startup: tile 0/1 inputs arrive as xs-only + z-only DMAs with the
    first weight slice ([Wq0,Wq1]) raced in between, so the first
    matmul fires ~3us earlier than with monolithic transfers.
  - Drain: the last tile's tanh+combine+store run in two half-tiles so
    the Scalar->Vector->DMA tail pipelines instead of serializing.
  - All elementwise work is bf16 (DVE 2x mode); the z->bf16 ScalarE
    cast of the baseline is gone.
"""

from contextlib import ExitStack

import ml_dtypes
import numpy as np

import concourse.bass as bass
import concourse.mybir as mybir
import concourse.tile as tile
from concourse import bacc
from concourse.bass_utils import run_bass_kernel_spmd

B, IN_DIM, WIDTH, L = 16, 512, 128, 8192
N_CORES = 8
B_PER = B // N_CORES          # batches per core
KC = IN_DIM // 128            # K chunks for the W matmuls
NT = 512                      # positions per tile
N_LT = L // NT                # position tiles per batch
T = B_PER * N_LT              # tiles per core
SS = 2                        # tiles per DMA superstep
XS_B = KC * NT * 2            # bf16 xs bytes per row
Z_B = 2 * NT                  # bf16 z bytes per row
ROW = XS_B + Z_B              # input row bytes per (partition, tile)

F32 = mybir.dt.float32
BF16 = mybir.dt.bfloat16
U8 = mybir.dt.uint8

_module_cache = {}


def _build():
    key = ("bf16v5", NT, SS)
    if key in _module_cache:
        return _module_cache[key]

    nc = bacc.Bacc("TRN2", target_bir_lowering=False, debug=False,
                   num_devices=N_CORES)

    inp_d = nc.dram_tensor("inp", [128, T, ROW], U8, kind="ExternalInput").ap()
    w_d = nc.dram_tensor("wall", [128, 15, 128], BF16,
                         kind="ExternalInput").ap()
    b_d = nc.dram_tensor("ball", [128, 3], F32, kind="ExternalInput").ap()
    out_d = nc.dram_tensor("out", [128, T, NT], BF16,
                           kind="ExternalOutput").ap()

    with tile.TileContext(nc) as tc, ExitStack() as ctx:
        wpool = ctx.enter_context(tc.tile_pool(name="weights", bufs=1))
        io = ctx.enter_context(tc.tile_pool(name="io", bufs=4))
        sig = ctx.enter_context(tc.tile_pool(name="sig", bufs=3))
        acts = ctx.enter_context(tc.tile_pool(name="acts", bufs=2))
        ost_p = ctx.enter_context(tc.tile_pool(name="ost", bufs=3))
        ps_q = ctx.enter_context(tc.tile_pool(name="ps_q", bufs=2,
                                              space="PSUM"))
        ps_r = ctx.enter_context(tc.tile_pool(name="ps_r", bufs=2,
                                              space="PSUM"))
        ps_zt = ctx.enter_context(tc.tile_pool(name="ps_zt", bufs=3,
                                               space="PSUM"))
        ps_wu = ctx.enter_context(tc.tile_pool(name="ps_wu", bufs=1,
                                               space="PSUM"))

        wall = wpool.tile([128, 15, 128], BF16, tag="wall")
        ball = wpool.tile([128, 3], F32, tag="ball")

        # PE p-state warmup: the PE runs its first ~3us at 1.2GHz. Burn
        # dummy matmuls on scratch tiles while the startup DMAs are in
        # flight so the first real matmul runs at full clock.
        scr_w = wpool.tile([128, 128], BF16, tag="scr_w")
        scr_x = wpool.tile([128, NT], BF16, tag="scr_x")
        nc.gpsimd.memset(scr_w[:], 0.0)
        nc.gpsimd.memset(scr_x[:], 0.0)
        wu_ps = ps_wu.tile([128, NT], F32, tag="wu")
        for i in range(5):
            nc.tensor.matmul(wu_ps[:], scr_w[:], scr_x[:],
                             start=(i == 0), stop=(i == 4))

        # DMA engines drain one global queue, so transfer ORDER is what
        # matters: all weights first (480KB, every stationary of tile 0),
        # then tile-0/1 xs on SP; the z slices and biases (small, needed
        # later) go via the Activation queue.
        nc.sync.dma_start(wall[:], w_d[:])
        warm = []
        for t in range(2):
            wt = io.tile([128, 1, ROW], U8, tag="io_w", name="warm")
            nc.sync.dma_start(wt[:, :, 0:XS_B], inp_d[:, t:t + 1, 0:XS_B])
            nc.scalar.dma_start(wt[:, :, XS_B:ROW],
                                inp_d[:, t:t + 1, XS_B:ROW])
            if t == 0:
                nc.scalar.dma_start(ball[:], b_d[:])
            warm.append(wt)

        carry = None
        ost = [None]

        def finish(c):
            """Trailing half of tile tc_ (Uz@(rz) matmul, tanh, combine,
            store), emitted during tile tc_+1 (after the loop for the
            last tile, split in halves to pipeline the drain)."""
            zt_ps, q_s, rz, z_v, tc_ = c
            nc.tensor.matmul(zt_ps[:], wall[:, 14, :], rz[:],
                             start=False, stop=True)
            sc, jc = divmod(tc_, SS)
            if jc == 0:
                ost[0] = ost_p.tile([128, SS, NT], BF16, tag="ost",
                                    name="ost")
            if tc_ < T - 1:
                halves = ((0, NT),)
            else:
                halves = tuple((h * NT // 4, (h + 1) * NT // 4)
                               for h in range(4))
            for h0, h1 in halves:
                zt_s = acts.tile([128, h1 - h0], BF16, tag=f"zt_s{h0}",
                                 name="zt_s")
                nc.scalar.activation(zt_s[:], zt_ps[:, h0:h1],
                                     mybir.ActivationFunctionType.Tanh,
                                     bias=ball[:, 2:3])
                # out = zt + q*(z - zt)
                diff = acts.tile([128, h1 - h0], BF16, tag=f"diff{h0}",
                                 name="diff")
                nc.vector.tensor_sub(diff[:], z_v[:, h0:h1], zt_s[:])
                prod = acts.tile([128, h1 - h0], BF16, tag=f"prod{h0}",
                                 name="prod")
                nc.vector.tensor_mul(prod[:], q_s[:, h0:h1], diff[:])
                nc.vector.tensor_add(ost[0][:, jc, h0:h1], zt_s[:], prod[:])
                if tc_ >= T - 2:
                    # drain: ship each piece as soon as it is ready
                    nc.scalar.dma_start(
                        out_d[:, tc_:tc_ + 1, h0:h1],
                        ost[0][:, jc:jc + 1, h0:h1])
            if jc == SS - 1 and tc_ < T - 2:
                # out-DMAs ride the Activation HWDGE queue so the SP queue
                # only carries input DMAs
                nc.scalar.dma_start(out_d[:, SS * sc:SS * (sc + 1), :],
                                    ost[0][:])

        io_t = None
        for t in range(T):
            s, j = divmod(t, SS)
            if t < 2:
                cur, cj = warm[t], 0
            else:
                if j == 0:
                    io_t = io.tile([128, SS, ROW], U8, tag="io")
                    nc.sync.dma_start(io_t[:],
                                      inp_d[:, SS * s:SS * (s + 1), :])
                cur, cj = io_t, j
            xs_v = cur[:, cj, 0:XS_B].bitcast(BF16).rearrange(
                "p (k n) -> p k n", k=KC)
            z_v = cur[:, cj, XS_B:ROW].bitcast(BF16)

            q_ps = ps_q.tile([128, NT], F32, tag="q")
            for k in range(KC):
                nc.tensor.matmul(q_ps[:], wall[:, k, :], xs_v[:, k, :],
                                 start=(k == 0), stop=False)
            nc.tensor.matmul(q_ps[:], wall[:, 4, :], z_v,
                             start=False, stop=True)
            r_ps = ps_r.tile([128, NT], F32, tag="r")
            for k in range(KC):
                nc.tensor.matmul(r_ps[:], wall[:, 5 + k, :], xs_v[:, k, :],
                                 start=(k == 0), stop=False)
            nc.tensor.matmul(r_ps[:], wall[:, 9, :], z_v,
                             start=False, stop=True)

            if carry is not None:
                finish(carry)
                carry = None

            zt_ps = ps_zt.tile([128, NT], F32, tag="zt")
            for k in range(KC):
                nc.tensor.matmul(zt_ps[:], wall[:, 10 + k, :], xs_v[:, k, :],
                                 start=(k == 0), stop=False)

            q_s = sig.tile([128, NT], BF16, tag="q_s")
            nc.scalar.activation(q_s[:], q_ps[:],
                                 mybir.ActivationFunctionType.Sigmoid,
                                 bias=ball[:, 0:1])
            r_s = acts.tile([128, NT], BF16, tag="r_s")
            nc.scalar.activation(r_s[:], r_ps[:],
                                 mybir.ActivationFunctionType.Sigmoid,
                                 bias=ball[:, 1:2])
            rz = acts.tile([128, NT], BF16, tag="rz")
            nc.vector.tensor_mul(rz[:], r_s[:], z_v)

            carry = (zt_ps, q_s, rz, z_v, t)

        finish(carry)

    nc.compile()
    _module_cache[key] = nc
    return nc


def _pack_inputs(xs, zp):
    """Per-core [128, T, ROW] uint8 rows: 4096B bf16 xs + 1024B bf16 z."""
    xsb = xs.astype(ml_dtypes.bfloat16)
    v = xsb.reshape(B, KC, 128, N_LT, NT)        # [b, k, p, i, n]
    v = np.ascontiguousarray(v.transpose(0, 2, 3, 1, 4))  # [b, p, i, k, n]
    v = v.reshape(B, 128, N_LT, KC * NT).view(np.uint8)
    zb = zp.astype(ml_dtypes.bfloat16).reshape(B, 128, N_LT, NT)
    zb = zb.view(np.uint8).reshape(B, 128, N_LT, Z_B)
    rows = np.concatenate([v, zb], axis=-1)      # [b, p, i, ROW]
    cores = []
    for c in range(N_CORES):
        rc = rows[c * B_PER:(c + 1) * B_PER]
        rc = rc.transpose(1, 0, 2, 3).reshape(128, T, ROW)
        cores.append(np.ascontiguousarray(rc))
    return cores


def _pack_weights(inputs):
    wall = np.zeros((128, 15, 128), dtype=np.float32)
    ball = np.zeros((128, 3), dtype=np.float32)
    for g, (wn, un, wbn, ubn) in enumerate((
            ("Wq_w", "Uq_w", "Wq_b", "Uq_b"),
            ("Wr_w", "Ur_w", "Wr_b", "Ur_b"),
            ("Wz_w", "Uz_w", "Wz_b", "Uz_b"))):
        W = np.asarray(inputs[wn], dtype=np.float32)         # [128, 512]
        U = np.asarray(inputs[un], dtype=np.float32)         # [128, 128]
        Wt = W.T.reshape(KC, 128, 128)                       # [k, p, m]
        for k in range(KC):
            wall[:, 5 * g + k, :] = Wt[k]
        wall[:, 5 * g + 4, :] = U.T
        ball[:, g] = (np.asarray(inputs[wbn], dtype=np.float32)
                      + np.asarray(inputs[ubn], dtype=np.float32))
    return (np.ascontiguousarray(wall.astype(ml_dtypes.bfloat16)),
            np.ascontiguousarray(ball))


def _run(inputs, trace=False, **run_kwargs):
    xs = np.asarray(inputs["xs"], dtype=np.float32)
    zp = np.ascontiguousarray(np.asarray(inputs["z_prev"], dtype=np.float32))
    assert xs.shape == (B, IN_DIM, L) and zp.shape == (B, WIDTH, L)

    inp_cores = _pack_inputs(xs, zp)
    wall, ball = _pack_weights(inputs)

    nc = _build()
    in_maps = [{"inp": inp_cores[c], "wall": wall, "ball": ball}
               for c in range(N_CORES)]

    res = run_bass_kernel_spmd(nc, in_maps, core_ids=list(range(N_CORES)),
                               trace=trace, **run_kwargs)
    outs = []
    for c in range(N_CORES):
        o = res.results[c]["out"]                 # [128, T, NT] bf16
        o = np.asarray(o).reshape(128, B_PER, N_LT, NT)
        o = o.transpose(1, 0, 2, 3).reshape(B_PER, 128, L)
        outs.append(o.astype(np.float32))
    out = np.concatenate(outs, axis=0)
    return out, res


def kernel(**inputs):
    out, _ = _run(inputs, trace=False)
    return out
